# revision 43
# baseline (speedup 1.0000x reference)
"""Multi-head causal attention on 8 TRN2 NeuronCores.

Sharding: core c -> batch c//2, head-group c%2 (8 of 16 heads).
Wq/Wk/Wv column-sharded, Wo row-sharded; the Wo all-reduce is the host-side
sum of the two partial outputs per batch.

Per-core kernel (Bass/Tile), bf16 data / f32 accumulation:
  Pipeline per sq-tile t (512 queries):
    attention(t) chunk loop [QK row-tiled head pairs -> fused exp on a
    2-bank PSUM slot -> causal zeroing via gpsimd affine_select -> PV with
    ones-column denominator], interleaved with K/V/Q projection units of
    tile t+1 and output-projection units of tile t-1 as PE filler so the
    tensor engine stays dense (HAM at full clock).
  Fully-masked columns of diagonal chunks are trimmed from QK/exp/select/PV.
  Softmax denominator reciprocal via the fast custom-DVE approx (the exact
  DVE reciprocal costs ~4us per [1,512]); per-query broadcast via a 1-row
  PE matmul; normalize on DVE directly into the aoT tile.
"""

import os
import sys

for _p in ("/opt/trn_rl_repo", "/root/.axon_site/_ro/trn_rl_repo"):
    if os.path.isdir(_p) and _p not in sys.path:
        sys.path.insert(0, _p)

import numpy as np
import ml_dtypes

import concourse.bass as bass  # noqa: F401
import concourse.tile as tile
from concourse import bacc, mybir
from concourse.bass_utils import run_bass_kernel_spmd
from concourse.dve_ops import RECIP_APPROX_FAST_CONSTS, RECIPROCAL_APPROX_FAST

F32 = mybir.dt.float32
F32R = mybir.dt.float32r
BF16 = mybir.dt.bfloat16
NPBF16 = ml_dtypes.bfloat16

# The kernel uses both Exp and Ln on the scalar engine. The act-table-load
# pass greedily picks the first function set containing each function, which
# thrashes between exp_and_others and natural_log_exp_and_others (a ~2.7us
# table reload per switch, ~53 reloads). Blank out every other set's function
# list (preserving list positions, which are the emitted set ids) so every
# activation resolves to the one set that contains both exp and ln.
import functools as _functools

from concourse import bacc as _bacc_mod
from concourse import hw_specs as _hw_specs

_ORIG_GAT = _hw_specs.get_activation_tables


@_functools.cache
def _gat_nat_log_exp_only(arch):
    t = _ORIG_GAT(arch)
    keep = "natural_log_exp_and_others"
    assert keep in t, sorted(t)
    return {k: (v if k == keep else set()) for k, v in t.items()}


_hw_specs.get_activation_tables = _gat_nat_log_exp_only
_bacc_mod.get_activation_tables = _gat_nat_log_exp_only

B, S, D = 4, 2048, 1024
H, DH = 16, 64
SCALE = DH**-0.5
NCORES = 8
NHPC = 8
HDPC = NHPC * DH  # 512
SQT = 512
NSQT = S // SQT  # 4
SKC = 128
NSKC = S // SKC  # 16
NDC = D // 128  # 8
NMC = HDPC // 128  # 4
NEG = -1.0e30

CFG = {
    "ebufs": 8,
    "psq": 3,
    "psv2": 1,  # fused [65, 2*SQT] pv-pair slots
    "xbufs": 3,
    "pvlag": 4,  # chunks the PV matmuls trail QK/exp by
}
DBG = False

LAST_RESULTS = None


def _mask_layout(mask: np.ndarray):
    """Blocks of [sk=128, sq=512].  Returns chunks[t] = list of
    (c, kind, arg): kind 'clear' (no masking), 'affine' (causal-style
    triangle, arg = affine base), or 'madd' (arg = packed additive tile idx).
    Fully-masked blocks are dropped."""
    chunks = []
    uniq = {}
    madds = []
    rr = np.arange(SKC)[:, None]
    jj = np.arange(SQT)[None, :]
    for t in range(NSQT):
        lst = []
        for c in range(NSKC):
            sub = mask[t * SQT : (t + 1) * SQT, c * SKC : (c + 1) * SKC]
            if sub.all():
                continue
            if not sub.any():
                lst.append((c, "clear", 0))
                continue
            subT = sub.T
            base = c * SKC - t * SQT
            if np.array_equal(subT, (rr + base) > jj):
                lst.append((c, "affine", base))
                continue
            key = subT.tobytes()
            if key not in uniq:
                madds.append(np.where(subT, NEG, 0.0).astype(np.float32))
                uniq[key] = len(madds) - 1
            lst.append((c, "madd", uniq[key]))
        assert lst, f"sq tile {t} fully masked"
        chunks.append(lst)
    madd_arr = (
        np.stack(madds) if madds else np.zeros((1, SKC, SQT), dtype=np.float32)
    )
    return chunks, madd_arr, bool(madds)


def _build_program(chunks, n_madd, use_madd, cfg):
    nc = bacc.Bacc(
        "TRN2", target_bir_lowering=False, debug=False, num_devices=NCORES
    )
    # tile-major host layouts: every DMA line is 8KB contiguous per partition
    xqT = nc.dram_tensor("xqT", [NSQT, 128, NDC * SQT], BF16, kind="ExternalInput").ap()
    xkT = nc.dram_tensor("xkT", [NSQT, 128, NDC * SQT], BF16, kind="ExternalInput").ap()
    xvT = nc.dram_tensor("xvT", [NSQT, 128, NDC * SQT], BF16, kind="ExternalInput").ap()
    wq = nc.dram_tensor("wq", [128, NDC * HDPC], BF16, kind="ExternalInput").ap()
    wk = nc.dram_tensor("wk", [128, NDC * HDPC], BF16, kind="ExternalInput").ap()
    wv = nc.dram_tensor("wv", [128, NDC * HDPC], BF16, kind="ExternalInput").ap()
    wo = nc.dram_tensor("wo", [128, NMC * D], BF16, kind="ExternalInput").ap()
    if use_madd:
        madd = nc.dram_tensor(
            "madd", [n_madd, SKC, SQT], F32, kind="ExternalInput"
        ).ap()
    out = nc.dram_tensor("out", [S, D], BF16, kind="ExternalOutput").ap()
    if DBG:
        dbg_kT = nc.dram_tensor("dbg_kT", [128, NMC, S], BF16, kind="ExternalOutput").ap()
        dbg_v = nc.dram_tensor("dbg_v", [128, NSKC, NHPC, DH + 1], BF16, kind="ExternalOutput").ap()
        dbg_qT = nc.dram_tensor("dbg_qT", [128, NMC, SQT], BF16, kind="ExternalOutput").ap()
        dbg_ao = nc.dram_tensor("dbg_ao", [128, NMC, SQT], BF16, kind="ExternalOutput").ap()
        dbg_den = nc.dram_tensor("dbg_den", [65, 4, SQT], F32, kind="ExternalOutput").ap()

    with tile.TileContext(nc) as tc:
        with (
            tc.tile_pool(name="const", bufs=1) as const,
            tc.tile_pool(name="xpool", bufs=cfg["xbufs"]) as xpool,
            tc.tile_pool(name="qtp", bufs=2) as qtp,
            tc.tile_pool(name="aop", bufs=2) as aop,
            tc.tile_pool(name="big", bufs=1) as big,
            tc.tile_pool(name="epool", bufs=cfg["ebufs"]) as epool,
            tc.tile_pool(name="spool", bufs=3) as spool,
            tc.tile_pool(name="opool", bufs=2) as opool,
            tc.tile_pool(name="psq", bufs=cfg["psq"], space="PSUM") as psq,
            tc.tile_pool(name="psv", bufs=cfg["psv2"], space="PSUM") as psv,
        ):
            # ---- constants ----
            ones_plane = const.tile([128, 128], F32)
            nc.vector.memset(ones_plane, 1.0)
            ones_r = const.tile([65, 64], F32R)
            o65f = const.tile([65, 64], F32)
            nc.vector.memset(o65f, 1.0)
            nc.vector.tensor_copy(ones_r, o65f)
            if use_madd:
                ident_sb = const.tile([128, 128], BF16)
                nc.gpsimd.memset(ident_sb, 0.0)
                nc.gpsimd.affine_select(
                    out=ident_sb,
                    in_=ident_sb,
                    compare_op=mybir.AluOpType.not_equal,
                    fill=1.0,
                    base=0,
                    pattern=[[-1, 128]],
                    channel_multiplier=1,
                )
                madd_sb = const.tile([SKC, n_madd, SQT], BF16)
                nc.gpsimd.dma_start(madd_sb, madd.rearrange("n p s -> p n s"))
            # startup loads spread across the three DMA-capable engines so
            # the first projection isn't gated on one serialized queue
            wq_sb = const.tile([128, NDC, HDPC], BF16)
            wk_sb = const.tile([128, NDC, HDPC], BF16)
            wv_sb = const.tile([128, NDC, HDPC], BF16)
            wo_sb = const.tile([128, NMC, D], BF16)
            # wk and xk0 gate the very first projection matmul: split each
            # into kc-halves across both HW DMA queues so they land sooner
            wk_r = wk.rearrange("p (c m) -> p c m", c=NDC)
            nc.sync.dma_start(wk_sb[:, 0 : NDC // 2, :], wk_r[:, 0 : NDC // 2, :])
            nc.scalar.dma_start(wk_sb[:, NDC // 2 :, :], wk_r[:, NDC // 2 :, :])

            # ---- persistent tiles ----
            kT_sb = big.tile([128, NMC, S], BF16, tag="kT")
            v_sb = big.tile([128, NSKC, NHPC, DH + 1], BF16, tag="v")
            nc.vector.tensor_copy(
                v_sb[:, :, :, DH : DH + 1],
                ones_plane.rearrange("p (a b c) -> p a b c", a=NSKC, b=NHPC),
            )

            def load_xT(src, n, eng=None):
                xt = xpool.tile([128, NDC, SQT], BF16, tag="xt")
                (eng or nc.sync).dma_start(
                    xt, src[n].rearrange("p (c s) -> p c s", c=NDC)
                )
                return xt

            # ---- projection units (each: one 2-bank psum slot + cast) ----
            def kproj_unit(xt, n, half):
                slot = psq.tile([128, 2 * SQT], F32, tag="qk2")
                for h2 in range(2):
                    m = 2 * half + h2
                    for kc in range(NDC):
                        nc.tensor.matmul(
                            slot[:, h2 * SQT : (h2 + 1) * SQT],
                            wk_sb[:, kc, m * 128 : (m + 1) * 128],
                            xt[:, kc, :],
                            start=(kc == 0),
                            stop=(kc == NDC - 1),
                        )
                nc.vector.tensor_copy(
                    kT_sb[:, 2 * half : 2 * half + 2, n * SQT : (n + 1) * SQT],
                    slot.rearrange("p (a s) -> p a s", a=2),
                )

            def qproj_unit(xt, qT_t, half):
                slot = psq.tile([128, 2 * SQT], F32, tag="qk2")
                for h2 in range(2):
                    m = 2 * half + h2
                    for kc in range(NDC):
                        nc.tensor.matmul(
                            slot[:, h2 * SQT : (h2 + 1) * SQT],
                            wq_sb[:, kc, m * 128 : (m + 1) * 128],
                            xt[:, kc, :],
                            start=(kc == 0),
                            stop=(kc == NDC - 1),
                        )
                nc.vector.tensor_copy(
                    qT_t[:, 2 * half : 2 * half + 2, :],
                    slot.rearrange("p (a s) -> p a s", a=2),
                )

            def vproj_unit(xt, n, half):
                slot = psq.tile([128, 2 * SQT], F32, tag="qk2")
                for h2 in range(2):
                    si = 2 * half + h2
                    for kc in range(NDC):
                        nc.tensor.matmul(
                            slot[:, h2 * SQT : (h2 + 1) * SQT],
                            xt[:, kc, si * 128 : (si + 1) * 128],
                            wv_sb[:, kc, :],
                            start=(kc == 0),
                            stop=(kc == NDC - 1),
                        )
                sc0 = n * 4 + 2 * half
                nc.vector.tensor_copy(
                    v_sb[:, sc0 : sc0 + 2, :, 0:DH],
                    slot.rearrange("p (a h e) -> p a h e", a=2, h=NHPC),
                )

            def outproj_sc(aoT_prev, sc):
                si = sc % 4
                slot = psq.tile([128, 2 * SQT], F32, tag="qk2")
                for j in range(2):
                    for mc2 in range(NMC):
                        nc.tensor.matmul(
                            slot[:, j * SQT : (j + 1) * SQT],
                            aoT_prev[:, mc2, si * 128 : (si + 1) * 128],
                            wo_sb[:, mc2, j * SQT : (j + 1) * SQT],
                            start=(mc2 == 0),
                            stop=(mc2 == NMC - 1),
                        )
                o_sb = opool.tile([128, 2 * SQT], BF16, tag="o")
                nc.vector.tensor_copy(o_sb, slot)
                nc.sync.dma_start(out[sc * 128 : (sc + 1) * 128, :], o_sb)

            # ---- epilogue: 1/den via fast approx, PE broadcast, normalize ----
            def epilogue(pv2, hp_, aoT_t_, dbg=False):
                # 1/den = exp(-ln(den)): one Ln over both heads' den rows,
                # PE broadcast of the ln values, then Exp(scale=-1) on the
                # broadcast -- all in the natural_log_exp_and_others set.
                lnr = spool.tile([65, 2 * SQT], F32R, tag="lnr")
                nc.scalar.activation(
                    lnr[64:65, :],
                    pv2[64:65, :],
                    mybir.ActivationFunctionType.Ln,
                )
                bc2 = psq.tile([128, 2 * SQT], F32, tag="qk2")
                nc.tensor.matmul(
                    bc2[0:64, 0:SQT],
                    ones_r[64:65, :],
                    lnr[64:65, 0:SQT],
                    start=True,
                    stop=True,
                )
                nc.tensor.matmul(
                    bc2[0:64, SQT : 2 * SQT],
                    ones_r[64:65, :],
                    lnr[64:65, SQT:],
                    start=True,
                    stop=True,
                )
                bcast = spool.tile([64, 2 * SQT], F32, tag="bcast")
                nc.scalar.activation(
                    bcast,
                    bc2[0:64, :],
                    mybir.ActivationFunctionType.Exp,
                    scale=-1.0,
                )
                nc.vector.tensor_mul(
                    aoT_t_[0:64, hp_, :], pv2[0:64, 0:SQT], bcast[:, 0:SQT]
                )
                tmpB = spool.tile([64, SQT], BF16, tag="tmpB")
                nc.vector.tensor_mul(tmpB, pv2[0:64, SQT:], bcast[:, SQT:])
                nc.sync.dma_start(aoT_t_[64:128, hp_, :], tmpB)
                if dbg:
                    dbg_sb = spool.tile([65, 4, SQT], F32, tag="dbg")
                    nc.vector.tensor_copy(dbg_sb[64:65, 0, :], pv2[64:65, 0:SQT])
                    nc.vector.tensor_copy(dbg_sb[0:1, 1, :], pv2[0:1, 0:SQT])
                    nc.vector.tensor_copy(dbg_sb[64:65, 2, :], lnr[64:65, 0:SQT])
                    nc.vector.tensor_copy(dbg_sb[0:1, 3, :], bcast[0:1, 0:SQT])
                    nc.sync.dma_start(dbg_den, dbg_sb)

            # ---- preamble: projections for tile 0 ----
            xk0 = xpool.tile([128, NDC, SQT], BF16, tag="xt")
            xk0_r = xkT[0].rearrange("p (c s) -> p c s", c=NDC)
            nc.sync.dma_start(xk0[:, 0 : NDC // 2, :], xk0_r[:, 0 : NDC // 2, :])
            nc.scalar.dma_start(xk0[:, NDC // 2 :, :], xk0_r[:, NDC // 2 :, :])
            xv0 = load_xT(xvT, 0)
            nc.scalar.dma_start(wv_sb, wv.rearrange("p (c m) -> p c m", c=NDC))
            xq0 = load_xT(xqT, 0, eng=nc.gpsimd)
            nc.gpsimd.dma_start(wq_sb, wq.rearrange("p (c m) -> p c m", c=NDC))
            nc.scalar.dma_start(wo_sb, wo.rearrange("p (c m) -> p c m", c=NMC))
            qT_t = qtp.tile([128, NMC, SQT], BF16, tag="qT")
            for half in range(2):
                kproj_unit(xk0, 0, half)
            for half in range(2):
                vproj_unit(xv0, 0, half)
            for half in range(2):
                qproj_unit(xq0, qT_t, half)

            # ---- main pipeline over sq tiles ----
            # epilogue EMISSION is deferred ~2 chunks into the next pair so
            # its Ln/Exp queue behind the next pair's first exps on ACT; the
            # single pv slot still serializes correctly via WAW deps, and
            # pvlag delays the next pair's first PV write past the epilogue.
            defer_epi = True
            prev_epi = None  # (pv2, hp, aoT_t)
            aoT_prev = None  # aoT of tile t-1 (for outproj filler)
            for t in range(NSQT):
                aoT_t = aop.tile([128, NMC, SQT], BF16, tag="aoT")
                qT_next = None

                # filler units: projections of t+1 first, then outproj(t-1)
                filler = []
                if t + 1 < NSQT:
                    xt_k = load_xT(xkT, t + 1)
                    xt_v = load_xT(xvT, t + 1)
                    xt_q = load_xT(xqT, t + 1)
                    qT_next = qtp.tile([128, NMC, SQT], BF16, tag="qT")
                    for half in range(2):
                        filler.append(
                            lambda h=half, x=xt_k, n=t + 1: kproj_unit(x, n, h)
                        )
                    for half in range(2):
                        filler.append(
                            lambda h=half, x=xt_v, n=t + 1: vproj_unit(x, n, h)
                        )
                    for half in range(2):
                        filler.append(
                            lambda h=half, x=xt_q, q=qT_next: qproj_unit(x, q, h)
                        )
                if aoT_prev is not None:
                    ao_ = aoT_prev
                    for si in range(4):
                        filler.append(
                            lambda s=si, a=ao_: outproj_sc(a, (t - 1) * 4 + s)
                        )

                n_chunks_t = len(chunks[t]) * (NHPC // 2)
                stride = max(2, n_chunks_t // (len(filler) + 1)) if filler else 0
                gidx = 0

                for hp in range(NHPC // 2):
                    hA, hB = 2 * hp, 2 * hp + 1
                    qsA = qT_t[0:64, hp, :]
                    qsB = qT_t[64:128, hp, :]
                    pv2 = psv.tile([65, 2 * SQT], F32, tag="pv")
                    pend = []  # PV lags QK/exp by up to 2 chunks
                    for ci, (c, kind, arg) in enumerate(chunks[t]):
                        tb = max(arg, 0) if kind == "affine" else 0
                        slot = psq.tile([128, 2 * SQT], F32, tag="qk2")
                        last = kind != "madd"
                        nc.tensor.matmul(
                            slot[:, tb:SQT],
                            kT_sb[0:64, hp, c * SKC : (c + 1) * SKC],
                            qsA[:, tb:SQT],
                            start=True,
                            stop=last,
                        )
                        nc.tensor.matmul(
                            slot[:, SQT + tb : 2 * SQT],
                            kT_sb[64:128, hp, c * SKC : (c + 1) * SKC],
                            qsB[:, tb:SQT],
                            start=True,
                            stop=last,
                        )
                        if kind == "madd":
                            for off in (0, SQT):
                                nc.tensor.matmul(
                                    slot[:, off : off + SQT],
                                    ident_sb,
                                    madd_sb[:, arg, :],
                                    start=False,
                                    stop=True,
                                )
                        e2 = epool.tile([128, 2 * SQT], BF16, tag="e")
                        if tb == 0:
                            nc.scalar.activation(
                                e2, slot, mybir.ActivationFunctionType.Exp
                            )
                        else:
                            nc.scalar.activation(
                                e2[:, tb:SQT],
                                slot[:, tb:SQT],
                                mybir.ActivationFunctionType.Exp,
                            )
                            nc.scalar.activation(
                                e2[:, SQT + tb :],
                                slot[:, SQT + tb :],
                                mybir.ActivationFunctionType.Exp,
                            )
                        if kind == "affine":
                            w = SQT - tb
                            ee = e2.rearrange("p (a s) -> p a s", a=2)[
                                :, :, tb:SQT
                            ]
                            nc.gpsimd.affine_select(
                                out=ee,
                                in_=ee,
                                compare_op=mybir.AluOpType.is_ge,
                                fill=0.0,
                                base=-(arg - tb),
                                pattern=[[0, 2], [1, w]],
                                channel_multiplier=-1,
                            )
                        if len(pend) == cfg["pvlag"]:
                            pc, pe2, ptb, pci = pend.pop(0)
                            nc.tensor.matmul(
                                pv2[:, ptb:SQT],
                                v_sb[:, pc, hA, :],
                                pe2[:, ptb:SQT],
                                start=(pci == 0),
                                stop=False,
                            )
                            nc.tensor.matmul(
                                pv2[:, SQT + ptb :],
                                v_sb[:, pc, hB, :],
                                pe2[:, SQT + ptb :],
                                start=(pci == 0),
                                stop=False,
                            )
                        pend.append((c, e2, tb, ci))
                        if ci == min(2, len(chunks[t]) - 1) and prev_epi is not None:
                            ppv, php, pao = prev_epi
                            epilogue(ppv, php, pao)
                            prev_epi = None
                        gidx += 1
                        if filler and stride and gidx % stride == 0:
                            filler.pop(0)()
                    while pend:
                        pc, pe2, ptb, pci = pend.pop(0)
                        nc.tensor.matmul(
                            pv2[:, ptb:SQT],
                            v_sb[:, pc, hA, :],
                            pe2[:, ptb:SQT],
                            start=(pci == 0),
                            stop=False if pend else True,
                        )
                        nc.tensor.matmul(
                            pv2[:, SQT + ptb :],
                            v_sb[:, pc, hB, :],
                            pe2[:, SQT + ptb :],
                            start=(pci == 0),
                            stop=False if pend else True,
                        )
                    if defer_epi:
                        prev_epi = (pv2, hp, aoT_t)
                    else:
                        epilogue(pv2, hp, aoT_t)
                while filler:
                    filler.pop(0)()
                aoT_prev = aoT_t
                if qT_next is not None:
                    qT_t = qT_next

            # flush: last pair epilogue + last tile's output projection
            if prev_epi is not None:
                ppv, php, pao = prev_epi
                epilogue(ppv, php, pao, dbg=DBG)
            for si in range(4):
                outproj_sc(aoT_prev, (NSQT - 1) * 4 + si)
            if DBG:
                nc.sync.dma_start(dbg_kT, kT_sb)
                nc.sync.dma_start(dbg_v, v_sb)
                nc.sync.dma_start(dbg_qT, qT_t)
                nc.sync.dma_start(dbg_ao, aoT_prev)

    nc.finalize()
    return nc


_PROG_CACHE = {}


def kernel(x_q, x_k, x_v, mask, Wq, Wk, Wv, Wo):
    global LAST_RESULTS
    x_q = np.asarray(x_q, dtype=np.float32)
    x_k = np.asarray(x_k, dtype=np.float32)
    x_v = np.asarray(x_v, dtype=np.float32)
    mask = np.asarray(mask).astype(bool)
    Wq = np.asarray(Wq, dtype=np.float32)
    Wk = np.asarray(Wk, dtype=np.float32)
    Wv = np.asarray(Wv, dtype=np.float32)
    Wo = np.asarray(Wo, dtype=np.float32)

    chunks, madd_arr, use_madd = _mask_layout(mask)
    key = (
        tuple(tuple(lst) for lst in chunks),
        madd_arr.shape[0],
        use_madd,
        tuple(sorted(CFG.items())),
    )
    if key not in _PROG_CACHE:
        _PROG_CACHE[key] = _build_program(
            chunks, madd_arr.shape[0], use_madd, CFG
        )
    nc = _PROG_CACHE[key]

    def tile_x(x):  # [S, D] -> [NSQT, 128, NDC*SQT], line-contiguous
        xT = x.T.astype(NPBF16)  # [D, S]
        return np.ascontiguousarray(
            xT.reshape(NDC, 128, NSQT, SQT)
            .transpose(2, 1, 0, 3)
            .reshape(NSQT, 128, NDC * SQT)
        )

    def tile_w(w):  # [D, HDPC] -> [128, NDC*HDPC]
        return np.ascontiguousarray(
            w.reshape(NDC, 128, HDPC).transpose(1, 0, 2).reshape(128, -1)
        )

    wq_s = (Wq * np.float32(SCALE)).astype(NPBF16)
    wk_b = Wk.astype(NPBF16)
    wv_b = Wv.astype(NPBF16)
    wo_b = Wo.astype(NPBF16)
    in_maps = []
    for c in range(NCORES):
        b = c // 2
        hs = slice((c % 2) * HDPC, (c % 2 + 1) * HDPC)
        wo_c = wo_b[hs, :]  # [HDPC, D]
        m = {
            "xqT": tile_x(x_q[b]),
            "xkT": tile_x(x_k[b]),
            "xvT": tile_x(x_v[b]),
            "wq": tile_w(wq_s[:, hs]),
            "wk": tile_w(wk_b[:, hs]),
            "wv": tile_w(wv_b[:, hs]),
            "wo": np.ascontiguousarray(
                wo_c.reshape(NMC, 128, D).transpose(1, 0, 2).reshape(128, -1)
            ),
        }
        if use_madd:
            m["madd"] = madd_arr
        in_maps.append(m)

    res = run_bass_kernel_spmd(nc, in_maps, core_ids=list(range(NCORES)))
    LAST_RESULTS = res
    out = np.empty((B, S, D), dtype=np.float32)
    for b in range(B):
        out[b] = res.results[2 * b]["out"].astype(np.float32) + res.results[
            2 * b + 1
        ]["out"].astype(np.float32)
    return out
